# revision 1
# baseline (speedup 1.0000x reference)
"""Trainium2 Bass kernel for nn_Classification_4922032521468.

Problem: acts = embeds[activity_index]  (A=512 rows, d=512)
         pairs = concat(acts[ii], acts[jj])  for all i<j (P=130816 pairs)
         out = log_softmax(pairs @ W.T + b)  -> [P, 4]

Key algebra: logits[p, c] = L[i, c] + R'[j, c]  with
  L  = acts @ Wl.T          (Wl = W[:, :512])
  R' = acts @ Wr.T + b      (Wr = W[:, 512:])
so log_softmax needs only lse[i, j] = ln(sum_c e^{L[i,c]} e^{R'[j,c]})
(a K=4 PE matmul of U = e^L rows against V = e^{R'}) and
  out[i, j, c] = L[i, c] + R'[j, c] - lse[i, j].
No 130816x1024 pair tensor is ever built.

Layout: the per-core output plane is computed TRANSPOSED - j on partitions,
(i, c) on the free axis - which makes every term either per-partition
(R', lse) or a partition-broadcast row (L, built once with a K=1 matmul).

Sharding: core k owns i-rows [64k, 64k+64). The same NEFF runs on all 8
cores (SPMD); per-core behavior comes only from per-core DATA:
activity_index is rotated by -64k so each core's own i-rows are gathered
rows 0..63. Each core outputs [512 j, 64 i, 4 c] (j rotated); the host
un-rotates j, transposes, and gathers the triu pairs.
"""

import numpy as np

A = 512  # number of activity tokens
D = 512  # embedding dim
C = 4  # classes
NTOK = 4096  # embeds table rows
RB = 64  # i-rows per core
NCORES = 8

_program = None
_last_results = None  # BassKernelResults from the most recent run (profiling)


def _build_program():
    from contextlib import ExitStack

    import concourse.bacc as bacc
    import concourse.mybir as mybir
    import concourse.tile as tile
    from concourse.bass import IndirectOffsetOnAxis
    from concourse.tile_rust import add_dep_helper

    fp32 = mybir.dt.float32
    i32 = mybir.dt.int32
    AF = mybir.ActivationFunctionType
    SUB = mybir.AluOpType.subtract
    ADD = mybir.AluOpType.add

    nc = bacc.Bacc(
        "TRN2",
        target_bir_lowering=False,
        debug=False,
        enable_asserts=False,
        num_devices=NCORES,
    )

    embeds_h = nc.dram_tensor("embeds", (NTOK, D), fp32, kind="ExternalInput")
    # idxs[p, j] = rotated activity_index[128j + p], int32
    idx_h = nc.dram_tensor("idxs", (128, 4), i32, kind="ExternalInput")
    # wt[d, 8k+0:4] = Wr.T[128k+d, :], wt[d, 8k+4:8] = Wl.T[128k+d, :]
    wt_h = nc.dram_tensor("wt", (128, 32), fp32, kind="ExternalInput")
    # b8 = [b_0..b_3, 0, 0, 0, 0] (bias folds into R via a K=1 matmul)
    b8_h = nc.dram_tensor("b8", (1, 8), fp32, kind="ExternalInput")
    # out[j, 4i + c] (j rotated per core)
    out_h = nc.dram_tensor("out", (A, RB * C), fp32, kind="ExternalOutput")

    ident_h = nc.inline_tensor(np.eye(128, dtype=np.float32), name="ident")

    embeds_ap = embeds_h.ap()
    out_ap = out_h.ap()

    with tile.TileContext(nc) as tc, ExitStack() as ctx:
        sb = ctx.enter_context(tc.tile_pool(name="sb", bufs=1))
        sbr = ctx.enter_context(tc.tile_pool(name="sbr", bufs=6))
        psT = ctx.enter_context(tc.tile_pool(name="psT", bufs=3, space="PSUM"))
        psR = ctx.enter_context(tc.tile_pool(name="psR", bufs=2, space="PSUM"))
        psB = ctx.enter_context(tc.tile_pool(name="psB", bufs=1, space="PSUM"))
        psS = ctx.enter_context(tc.tile_pool(name="psS", bufs=1, space="PSUM"))

        # ---- gather path first: idx load, then the 4 indirect gathers ----
        idxs = sb.tile([128, 4], i32, tag="idxs")
        nc.sync.dma_start(out=idxs[:], in_=idx_h.ap()[:])

        acts = []
        for j in range(4):
            aj = sb.tile([128, D], fp32, tag=f"acts{j}", name=f"acts{j}")
            nc.gpsimd.indirect_dma_start(
                out=aj[:],
                out_offset=None,
                in_=embeds_ap[:],
                in_offset=IndirectOffsetOnAxis(ap=idxs[:, j : j + 1], axis=0),
            )
            acts.append(aj)

        # ---- small constants (dispatch behind idx on the sync queue) ----
        ident = sb.tile([128, 128], fp32, tag="ident")
        nc.sync.dma_start(out=ident[:], in_=ident_h.ap()[:])
        wt = sb.tile([128, 32], fp32, tag="wt")
        nc.sync.dma_start(out=wt[:], in_=wt_h.ap()[:])
        b4 = sb.tile([C, 1], fp32, tag="b4")
        nc.sync.dma_start(out=b4[:], in_=b8_h.ap()[0:1, 0:C])
        ones = sb.tile([1, 128], fp32, tag="ones")
        nc.vector.memset(ones[:], 1.0)

        # persistent tiles
        rj = sb.tile([128, 16], fp32, tag="rj")  # R' row-major, chunk j cols 4j:4j+4
        rt = sb.tile([C, A], fp32, tag="rt")  # R' transposed
        vt = sb.tile([C, A], fp32, tag="vt")  # e^{R'} transposed (classes on K)
        ut4 = sb.tile([C, RB], fp32, tag="ut4")  # e^{L} transposed
        lt4 = sb.tile([C, RB], fp32, tag="lt4")  # L transposed
        lbf = sb.tile([1, RB * C], fp32, tag="lbf")  # L flattened (4i + c)

        # ---- phase A per j-chunk: transpose, R' matmuls, e^{R'} ----
        # (all Exp ops are emitted before any Ln so the ACT table loads once
        # per function instead of thrashing Exp<->Ln. Matmuls keep the tiny
        # wt as the STATIONARY operand - a [128, 128] stationary would pay a
        # ~1.3us weight load per call.)
        for j in range(4):
            aT = []
            for k in range(4):
                pt = psT.tile([128, 128], fp32, tag="pt", name="pt")
                nc.tensor.transpose(
                    out=pt[:],
                    in_=acts[j][:, 128 * k : 128 * k + 128],
                    identity=ident[:],
                )
                at = sbr.tile([128, 128], fp32, tag="aT", name="aT")
                nc.vector.tensor_copy(out=at[:], in_=pt[:])
                aT.append(at)

            # R'^T chunk [4, 128] = sum_k Wr.T_k.T @ aT_k  (+ b outer ones)
            pr = psR.tile([C, 128], fp32, tag="pr", name="pr")
            for k in range(4):
                nc.tensor.matmul(
                    out=pr[:],
                    lhsT=wt[:, 8 * k : 8 * k + 4],
                    rhs=aT[k][:],
                    start=(k == 0),
                    stop=(k == 3),
                )
            # b rides for free: ACT bias on the exp, DVE scalar-add on rt
            # (classes sit on partitions here, so b is a [4, 1] per-partition
            # operand) - no K=1 PE matmul needed.
            nc.vector.tensor_scalar_add(
                rt[:, 128 * j : 128 * (j + 1)], pr[:], b4[:]
            )
            last_exp = nc.scalar.activation(
                out=vt[:, 128 * j : 128 * (j + 1)],
                in_=pr[:],
                func=AF.Exp,
                bias=b4[:],
            )
            # row-major chunk for the final per-partition add
            prj = psT.tile([128, C], fp32, tag="pt", name="prj")
            nc.tensor.transpose(
                out=prj[:],
                in_=rt[:, 128 * j : 128 * (j + 1)],
                identity=ident[0:C, 0:C],
            )
            nc.vector.tensor_copy(out=rj[:, 4 * j : 4 * j + 4], in_=prj[:])

            if j == 0:
                # L^T [4, 64] (no bias; b lives on the R side)
                pl = psR.tile([C, RB], fp32, tag="pl", name="pl", bufs=1)
                for k in range(4):
                    nc.tensor.matmul(
                        out=pl[:],
                        lhsT=wt[:, 8 * k + 4 : 8 * k + 8],
                        rhs=aT[k][:, 0:RB],
                        start=(k == 0),
                        stop=(k == 3),
                    )
                nc.scalar.activation(out=ut4[:], in_=pl[:], func=AF.Exp)
                nc.vector.tensor_copy(out=lt4[:], in_=pl[:])
                # lbf[0, 4i+c] = L[i, c] via per-class reordering DMAs
                lbf3 = lbf[:].rearrange("o (i c) -> o i c", c=C)
                for c in range(C):
                    nc.sync.dma_start(
                        out=lbf3[:, :, c : c + 1], in_=lt4[c : c + 1, :]
                    )

        # L broadcast across all 128 partitions via K=1 matmul (kept in PSUM)
        lbb = psB.tile([128, RB * C], fp32, tag="lbb")
        nc.tensor.matmul(out=lbb[:], lhsT=ones[:], rhs=lbf[:], start=True, stop=True)
        lbb3 = lbb[:].rearrange("p (i c) -> p i c", c=C)

        # ---- phase B per j-chunk: lse, combine, store ----
        for j in range(4):
            se = psS.tile([128, RB], fp32, tag="se", name="se")
            nc.tensor.matmul(
                out=se[:],
                lhsT=vt[:, 128 * j : 128 * (j + 1)],
                rhs=ut4[:],
                start=True,
                stop=True,
            )
            lnse = sbr.tile([128, RB], fp32, tag="lnse", name="lnse")
            ln_inst = nc.scalar.activation(out=lnse[:], in_=se[:], func=AF.Ln)
            # keep every Ln after the last Exp so the ACT function table
            # loads exactly twice instead of thrashing Exp<->Ln per chunk
            add_dep_helper(
                ln_inst.ins, last_exp.ins, sync=False, reason="act-table order"
            )

            tmp = sbr.tile([128, RB * C], fp32, tag="tmp", name="tmp")
            nc.vector.tensor_tensor(
                out=tmp[:].rearrange("p (i c) -> p i c", c=C),
                in0=lbb3,
                in1=lnse[:].unsqueeze(2).to_broadcast([128, RB, C]),
                op=SUB,
            )
            oj = sbr.tile([128, RB * C], fp32, tag="oj", name="oj")
            nc.vector.tensor_tensor(
                out=oj[:].rearrange("p (i c) -> p i c", c=C),
                in0=tmp[:].rearrange("p (i c) -> p i c", c=C),
                in1=rj[:, 4 * j : 4 * j + 4].unsqueeze(1).to_broadcast([128, RB, C]),
                op=ADD,
            )
            nc.sync.dma_start(
                out=out_ap[128 * j : 128 * (j + 1), :], in_=oj[:]
            )

    nc.compile()
    return nc


def _get_program():
    global _program
    if _program is None:
        _program = _build_program()
    return _program


def _prep_core_inputs(embeds, idx64, wt_np, b8_np, k):
    rot = np.roll(idx64, -RB * k)
    idxs = np.ascontiguousarray(rot.reshape(4, 128).T.astype(np.int32))
    return {"embeds": embeds, "idxs": idxs, "wt": wt_np, "b8": b8_np}


def kernel(embeds, activity_index, W, b):
    from concourse.bass_utils import run_bass_kernel_spmd

    embeds = np.ascontiguousarray(np.asarray(embeds), dtype=np.float32)
    W = np.asarray(W, dtype=np.float32)
    b_in = np.asarray(b, dtype=np.float32).reshape(C)
    idx64 = np.asarray(activity_index).astype(np.int64)

    # wt[d, 8k+0:4] = Wr.T chunk k, wt[d, 8k+4:8] = Wl.T chunk k
    wt_np = np.empty((128, 32), dtype=np.float32)
    for k in range(4):
        wt_np[:, 8 * k : 8 * k + 4] = W[:, D + 128 * k : D + 128 * (k + 1)].T
        wt_np[:, 8 * k + 4 : 8 * k + 8] = W[:, 128 * k : 128 * (k + 1)].T
    wt_np = np.ascontiguousarray(wt_np)
    b8_np = np.zeros((1, 8), dtype=np.float32)
    b8_np[0, 0:C] = b_in

    nc = _get_program()
    in_maps = [
        _prep_core_inputs(embeds, idx64, wt_np, b8_np, k) for k in range(NCORES)
    ]

    results = run_bass_kernel_spmd(nc, in_maps, core_ids=list(range(NCORES)))
    global _last_results
    _last_results = results

    out_sq = np.empty((A, A, C), dtype=np.float32)
    for k in range(NCORES):
        # blk[j, i, c] with j rotated by -64k -> un-rotate and transpose
        blk = results.results[k]["out"].reshape(A, RB, C).transpose(1, 0, 2)
        out_sq[RB * k : RB * (k + 1)] = np.roll(blk, RB * k, axis=1)

    ii, jj = np.triu_indices(A, k=1)
    return np.ascontiguousarray(out_sq[ii, jj])



# revision 5
# speedup vs baseline: 1.0533x; 1.0533x over previous
"""Trainium2 Bass kernel for nn_Classification_4922032521468.

Problem: acts = embeds[activity_index]  (A=512 rows, d=512)
         pairs = concat(acts[ii], acts[jj])  for all i<j (P=130816 pairs)
         out = log_softmax(pairs @ W.T + b)  -> [P, 4]

Key algebra: logits[p, c] = L[i, c] + R'[j, c]  with
  L  = acts @ Wl.T          (Wl = W[:, :512])
  R' = acts @ Wr.T + b      (Wr = W[:, 512:])
so log_softmax needs only lse[i, j] = ln(sum_c e^{L[i,c]} e^{R'[j,c]})
(a K=4 PE matmul of U = e^L rows against V = e^{R'}) and
  out[i, j, c] = L[i, c] + R'[j, c] - lse[i, j].
No 130816x1024 pair tensor is ever built.

v2 layout/speed notes vs the first working kernel:
- embeds table, gathers, transposes and the d-contraction matmuls run in
  fp16 (halves gather bytes, 4x faster PE transposes); all accumulation,
  exp/ln and the output stay fp32. Error budget ~1e-3 rel << 2e-2 gate.
- One [4, 512] R'^T and one [4, 64] L^T matmul group (wt stationary).
- The logits plane M[j, (i,c)] = L[i,c] + R'[j,c] is built by a K=4
  PE matmul pair (delta-tile trick) instead of DVE broadcasts, so the
  post-Ln work is a single tensor_tensor per j-chunk.
- DMAs spread across queues: idx+outs on sync, consts on scalar/vector,
  gathers on gpsimd. Phase B is chunk-pipelined (lse -> Ln -> sub -> DMA).

Sharding: core k owns i-rows [64k, 64k+64). The same NEFF runs on all 8
cores (SPMD); per-core behavior comes only from per-core DATA:
activity_index is rotated by -64k so each core's own i-rows are gathered
rows 0..63. Each core outputs [512 j, 64 i, 4 c] (j rotated); the host
un-rotates j, transposes, and gathers the triu pairs.
"""

import numpy as np

A = 512  # number of activity tokens
D = 512  # embedding dim
C = 4  # classes
NTOK = 4096  # embeds table rows
RB = 64  # i-rows per core
NCORES = 8

_program = None
_last_results = None  # BassKernelResults from the most recent run (profiling)


def _build_program():
    from contextlib import ExitStack

    import concourse.bacc as bacc
    import concourse.mybir as mybir
    import concourse.tile as tile
    from concourse.bass import IndirectOffsetOnAxis
    from concourse.tile_rust import add_dep_helper

    fp32 = mybir.dt.float32
    fp16 = mybir.dt.float16
    i32 = mybir.dt.int32
    AF = mybir.ActivationFunctionType
    SUB = mybir.AluOpType.subtract
    MULT = mybir.AluOpType.mult

    nc = bacc.Bacc(
        "TRN2",
        target_bir_lowering=False,
        debug=False,
        enable_asserts=False,
        num_devices=NCORES,
    )

    emb16_h = nc.dram_tensor("emb16", (NTOK, D), fp16, kind="ExternalInput")
    # idxs[p, j] = rotated activity_index[128j + p], int32
    idx_h = nc.dram_tensor("idxs", (128, 4), i32, kind="ExternalInput")
    # wt16[d, 8k+0:4] = Wr.T[128k+d, :], wt16[d, 8k+4:8] = Wl.T[128k+d, :]
    wt_h = nc.dram_tensor("wt16", (128, 32), fp16, kind="ExternalInput")
    b4_h = nc.dram_tensor("b4", (C, 1), fp32, kind="ExternalInput")
    # out[j, 4i + c] (j rotated per core)
    out_h = nc.dram_tensor("out", (A, RB * C), fp32, kind="ExternalOutput")

    ident_h = nc.inline_tensor(np.eye(128, dtype=np.float16), name="ident16")
    # cols 0:256 = tile(eye(4), 64) (delta pattern over (i, c)); 256:260 = eye(4)
    ydel_np = np.concatenate(
        [np.tile(np.eye(4, dtype=np.float16), 64), np.eye(4, dtype=np.float16)],
        axis=1,
    )
    ydel_h = nc.inline_tensor(ydel_np, name="ydel16")

    emb_ap = emb16_h.ap()
    out_ap = out_h.ap()

    with tile.TileContext(nc) as tc, ExitStack() as ctx:
        sb = ctx.enter_context(tc.tile_pool(name="sb", bufs=1))
        psT = ctx.enter_context(tc.tile_pool(name="psT", bufs=2, space="PSUM"))
        psR = ctx.enter_context(tc.tile_pool(name="psR", bufs=1, space="PSUM"))
        psM = ctx.enter_context(tc.tile_pool(name="psM", bufs=2, space="PSUM"))
        psS = ctx.enter_context(tc.tile_pool(name="psS", bufs=2, space="PSUM"))

        # ---- gather path first: idx load, then the 4 indirect gathers ----
        idxs = sb.tile([128, 4], i32, tag="idxs")
        nc.sync.dma_start(out=idxs[:], in_=idx_h.ap()[:])

        acts = []
        for j in range(4):
            aj = sb.tile([128, D], fp16, tag=f"acts{j}", name=f"acts{j}")
            nc.gpsimd.indirect_dma_start(
                out=aj[:],
                out_offset=None,
                in_=emb_ap[:],
                in_offset=IndirectOffsetOnAxis(ap=idxs[:, j : j + 1], axis=0),
            )
            acts.append(aj)

        # ---- small constants on other queues (parallel with idx/gathers) ----
        wt = sb.tile([128, 32], fp16, tag="wt")
        nc.scalar.dma_start(out=wt[:], in_=wt_h.ap()[:])
        ident = sb.tile([128, 128], fp16, tag="ident")
        nc.scalar.dma_start(out=ident[:], in_=ident_h.ap()[:])
        ydel = sb.tile([4, 260], fp16, tag="ydel")
        nc.sync.dma_start(out=ydel[:], in_=ydel_h.ap()[:])
        b4 = sb.tile([C, 1], fp32, tag="b4")
        nc.sync.dma_start(out=b4[:], in_=b4_h.ap()[:])
        ones4 = sb.tile([C, 128], fp16, tag="ones4")
        nc.vector.memset(ones4[:], 1.0)

        # persistent tiles
        aT = [sb.tile([128, D], fp16, tag=f"aT{k}", name=f"aT{k}") for k in range(4)]
        vt = sb.tile([C, A], fp32, tag="vt")  # e^{R'+b} transposed (classes on K)
        ut4 = sb.tile([C, RB], fp32, tag="ut4")  # e^{L} transposed
        x4 = sb.tile([C, A], fp16, tag="x4")  # R'+b transposed, fp16
        yb = sb.tile([C, RB * C], fp16, tag="yb")  # L[i,c]*delta[c',c] over (i,c)
        lnse = sb.tile([128, RB * 4], fp32, tag="lnse")  # ln(se), col 64j per chunk
        oj = sb.tile([128, RB * C * 4], fp32, tag="oj")  # output, 256 cols per chunk

        # ---- transposes: acts[j][:, 128k:+128] -> aT[k][:, 128j:+128] ----
        # chunk 0 first, then its L^T matmuls slot into PE idle time while
        # the remaining gathers land.
        prL = psR.tile([C, RB], fp32, tag="prL", name="prL")
        prR = psR.tile([C, A], fp32, tag="prR", name="prR", bufs=1)

        def do_chunk_transposes(j):
            for k in range(4):
                pt = psT.tile([128, 128], fp16, tag="pt", name="pt")
                nc.tensor.transpose(
                    out=pt[:],
                    in_=acts[j][:, 128 * k : 128 * k + 128],
                    identity=ident[:],
                )
                nc.vector.tensor_copy(
                    out=aT[k][:, 128 * j : 128 * j + 128], in_=pt[:]
                )

        do_chunk_transposes(0)
        # L^T [4, 64]: only needs chunk-0 columns of each aT[k]
        for k in range(4):
            nc.tensor.matmul(
                out=prL[:],
                lhsT=wt[:, 8 * k + 4 : 8 * k + 8],
                rhs=aT[k][:, 0:RB],
                start=(k == 0),
                stop=(k == 3),
            )
        for j in range(1, 4):
            do_chunk_transposes(j)
        # R'^T [4, 512] accumulated over the 4 d-chunks
        for k in range(4):
            nc.tensor.matmul(
                out=prR[:],
                lhsT=wt[:, 8 * k : 8 * k + 4],
                rhs=aT[k][:],
                start=(k == 0),
                stop=(k == 3),
            )

        # ---- activations + small combines ----
        expU = nc.scalar.activation(out=ut4[:], in_=prL[:], func=AF.Exp)
        expV = nc.scalar.activation(out=vt[:], in_=prR[:], func=AF.Exp, bias=b4[:])
        # X4 = (R' + b) as fp16 row for the M matmul
        nc.vector.tensor_scalar_add(x4[:], prR[:], b4[:])
        # Yb[c', (i,c)] = L^T[c', i] * delta[c', c]
        nc.vector.tensor_tensor(
            out=yb[:].rearrange("p (i c) -> p i c", c=C),
            in0=prL[:].unsqueeze(2).to_broadcast([C, RB, C]),
            in1=ydel[:, 256:260].unsqueeze(1).to_broadcast([C, RB, C]),
            op=MULT,
        )

        # ---- phase B per j-chunk: lse, M, ln, subtract, store ----
        last_exp = expV
        for j in range(4):
            se = psS.tile([128, RB], fp32, tag="se", name="se")
            nc.tensor.matmul(
                out=se[:],
                lhsT=vt[:, 128 * j : 128 * (j + 1)],
                rhs=ut4[:],
                start=True,
                stop=True,
            )
            # M[j', (i,c)] = (R'+b)[128j+j', c] + L[i, c]
            mj = psM.tile([128, RB * C], fp32, tag="mj", name="mj")
            nc.tensor.matmul(
                out=mj[:],
                lhsT=x4[:, 128 * j : 128 * (j + 1)],
                rhs=ydel[:, 0:256],
                start=True,
                stop=False,
            )
            nc.tensor.matmul(
                out=mj[:],
                lhsT=ones4[:],
                rhs=yb[:],
                start=False,
                stop=True,
            )
            ln_inst = nc.scalar.activation(
                out=lnse[:, RB * j : RB * (j + 1)], in_=se[:], func=AF.Ln
            )
            if j == 0:
                # keep every Ln after the last Exp so the ACT function table
                # loads exactly twice instead of thrashing Exp<->Ln
                add_dep_helper(
                    ln_inst.ins, last_exp.ins, sync=False, reason="act-table order"
                )
            nc.vector.tensor_tensor(
                out=oj[:, 256 * j : 256 * (j + 1)].rearrange(
                    "p (i c) -> p i c", c=C
                ),
                in0=mj[:].rearrange("p (i c) -> p i c", c=C),
                in1=lnse[:, RB * j : RB * (j + 1)]
                .unsqueeze(2)
                .to_broadcast([128, RB, C]),
                op=SUB,
            )
            eng = nc.sync if j % 2 == 0 else nc.scalar
            eng.dma_start(
                out=out_ap[128 * j : 128 * (j + 1), :],
                in_=oj[:, 256 * j : 256 * (j + 1)],
            )

    nc.compile()
    return nc


def _get_program():
    global _program
    if _program is None:
        _program = _build_program()
    return _program


def _prep_core_inputs(emb16, idx64, wt_np, b4_np, k):
    rot = np.roll(idx64, -RB * k)
    idxs = np.ascontiguousarray(rot.reshape(4, 128).T.astype(np.int32))
    return {"emb16": emb16, "idxs": idxs, "wt16": wt_np, "b4": b4_np}


def kernel(embeds, activity_index, W, b):
    from concourse.bass_utils import run_bass_kernel_spmd

    embeds = np.asarray(embeds, dtype=np.float32)
    emb16 = np.ascontiguousarray(embeds.astype(np.float16))
    W = np.asarray(W, dtype=np.float32)
    b_in = np.asarray(b, dtype=np.float32).reshape(C)
    idx64 = np.asarray(activity_index).astype(np.int64)

    # wt16[d, 8k+0:4] = Wr.T chunk k, wt16[d, 8k+4:8] = Wl.T chunk k
    wt_np = np.empty((128, 32), dtype=np.float16)
    for k in range(4):
        wt_np[:, 8 * k : 8 * k + 4] = W[:, D + 128 * k : D + 128 * (k + 1)].T
        wt_np[:, 8 * k + 4 : 8 * k + 8] = W[:, 128 * k : 128 * (k + 1)].T
    wt_np = np.ascontiguousarray(wt_np)
    b4_np = np.ascontiguousarray(b_in.reshape(C, 1))

    nc = _get_program()
    in_maps = [
        _prep_core_inputs(emb16, idx64, wt_np, b4_np, k) for k in range(NCORES)
    ]

    results = run_bass_kernel_spmd(nc, in_maps, core_ids=list(range(NCORES)))
    global _last_results
    _last_results = results

    out_sq = np.empty((A, A, C), dtype=np.float32)
    for k in range(NCORES):
        # blk[j, i, c] with j rotated by -64k -> un-rotate and transpose
        blk = results.results[k]["out"].reshape(A, RB, C).transpose(1, 0, 2)
        out_sq[RB * k : RB * (k + 1)] = np.roll(blk, RB * k, axis=1)

    ii, jj = np.triu_indices(A, k=1)
    return np.ascontiguousarray(out_sq[ii, jj])


# revision 12
# speedup vs baseline: 1.1198x; 1.0632x over previous
"""Trainium2 Bass kernel for nn_Classification_4922032521468.

Problem: acts = embeds[activity_index]  (A=512 rows, d=512)
         pairs = concat(acts[ii], acts[jj])  for all i<j (P=130816 pairs)
         out = log_softmax(pairs @ W.T + b)  -> [P, 4]

Key algebra: logits[p, c] = L[i, c] + R'[j, c]  with
  L  = acts @ Wl.T          (Wl = W[:, :512])
  R' = acts @ Wr.T + b      (Wr = W[:, 512:])
so log_softmax needs only lse[i, j] = ln(sum_c e^{L[i,c]} e^{R'[j,c]})
(a K=4 PE matmul of U = e^L rows against V = e^{R'}) and
  out[i, j, c] = L[i, c] + R'[j, c] - lse[i, j].
No 130816x1024 pair tensor is ever built.

v3 speed notes:
- fp16 input path (gather/transpose/d-contraction); fp32 accum + output.
- One [128, 8] stationary computes R'^T and L^T together: pr [8, 512].
- A dummy Ln is issued first so the ACT table pass picks the
  natural_log_exp_and_others set (holds BOTH exp and ln): one table load
  total, early, and no exp->ln ordering constraint in phase B.
- vt/ut in fp16 so each lse matmul is a single stationary load.
- Logits plane M[j, (i,c)] = L[i,c] + R'[j,c] via ONE K=8 matmul per
  chunk (delta-tile trick; x8 rows 4-7 are ones, Y8 rows 4-7 = L*delta).
- Phase B chunk-pipelined: lse -> ln -> (M) -> subtract -> DMA out.

Sharding: core k owns i-rows [64k, 64k+64). The same NEFF runs on all 8
cores (SPMD); per-core behavior comes only from per-core DATA:
activity_index is rotated by -64k so each core's own i-rows are gathered
rows 0..63. Each core outputs [512 j, 64 i, 4 c] (j rotated); the host
un-rotates j, transposes, and gathers the triu pairs.
"""

import numpy as np

A = 512  # number of activity tokens
D = 512  # embedding dim
C = 4  # classes
NTOK = 4096  # embeds table rows
RB = 64  # i-rows per core
NCORES = 8

USE_DRAM_IDX = False  # HW requires gather offsets resident in SBUF

_program = None
_last_results = None  # BassKernelResults from the most recent run (profiling)


def _build_program():
    from contextlib import ExitStack

    import concourse.bacc as bacc
    import concourse.mybir as mybir
    import concourse.tile as tile
    from concourse.bass import IndirectOffsetOnAxis

    fp32 = mybir.dt.float32
    fp16 = mybir.dt.float16
    i32 = mybir.dt.int32
    AF = mybir.ActivationFunctionType
    SUB = mybir.AluOpType.subtract
    MULT = mybir.AluOpType.mult

    nc = bacc.Bacc(
        "TRN2",
        target_bir_lowering=False,
        debug=False,
        enable_asserts=False,
        num_devices=NCORES,
    )

    emb16_h = nc.dram_tensor("emb16", (NTOK, D), fp16, kind="ExternalInput")
    # idxs[p, j] = rotated activity_index[128j + p], int32
    idx_h = nc.dram_tensor("idxs", (128, 4), i32, kind="ExternalInput")
    # wt16[d, 8k+0:4] = Wr.T[128k+d, :], wt16[d, 8k+4:8] = Wl.T[128k+d, :]
    wt_h = nc.dram_tensor("wt16", (128, 32), fp16, kind="ExternalInput")
    b4_h = nc.dram_tensor("b4", (C, 1), fp32, kind="ExternalInput")
    # out[j, 4i + c] (j rotated per core)
    out_h = nc.dram_tensor("out", (A, RB * C), fp32, kind="ExternalOutput")

    ident_h = nc.inline_tensor(np.eye(128, dtype=np.float16), name="ident16")
    # rows 0-3: cols 0:256 = tile(eye(4), 64), cols 256:260 = eye(4)
    ydel_np = np.zeros((8, 260), dtype=np.float16)
    ydel_np[0:4, 0:256] = np.tile(np.eye(4, dtype=np.float16), 64)
    ydel_np[0:4, 256:260] = np.eye(4, dtype=np.float16)
    ydel_h = nc.inline_tensor(ydel_np, name="ydel16")

    emb_ap = emb16_h.ap()
    out_ap = out_h.ap()

    with tile.TileContext(nc) as tc, ExitStack() as ctx:
        sb = ctx.enter_context(tc.tile_pool(name="sb", bufs=1))
        psT = ctx.enter_context(tc.tile_pool(name="psT", bufs=2, space="PSUM"))
        psR = ctx.enter_context(tc.tile_pool(name="psR", bufs=1, space="PSUM"))
        psM = ctx.enter_context(tc.tile_pool(name="psM", bufs=2, space="PSUM"))
        psS = ctx.enter_context(tc.tile_pool(name="psS", bufs=2, space="PSUM"))

        # dummy Ln first: forces the ACT pass to load the combined
        # natural_log_exp_and_others table set (ln only exists there), so
        # the later Exp calls reuse it and phase B's Ln needs no reload.
        dmy = sb.tile([1, 8], fp32, tag="dmy")
        nc.gpsimd.memset(dmy[:], 1.0)
        dmy2 = sb.tile([1, 8], fp32, tag="dmy2")
        nc.scalar.activation(out=dmy2[:], in_=dmy[:], func=AF.Ln)

        # ---- gather path ----
        if USE_DRAM_IDX:
            idx_off = idx_h.ap()
        else:
            idxs = sb.tile([128, 4], i32, tag="idxs")
            nc.sync.dma_start(out=idxs[:], in_=idx_h.ap()[:])
            idx_off = idxs

        acts = []
        for j in range(4):
            aj = sb.tile([128, D], fp16, tag=f"acts{j}", name=f"acts{j}")
            nc.gpsimd.indirect_dma_start(
                out=aj[:],
                out_offset=None,
                in_=emb_ap[:],
                in_offset=IndirectOffsetOnAxis(ap=idx_off[:, j : j + 1], axis=0),
            )
            acts.append(aj)

        # ---- small constants on sync/scalar queues (parallel w/ gathers) ----
        wt = sb.tile([128, 32], fp16, tag="wt")
        nc.scalar.dma_start(out=wt[:], in_=wt_h.ap()[:])
        ident = sb.tile([128, 128], fp16, tag="ident")
        nc.scalar.dma_start(out=ident[:], in_=ident_h.ap()[:])
        y8 = sb.tile([8, 256], fp16, tag="y8")
        nc.sync.dma_start(out=y8[0:4, :], in_=ydel_h.ap()[0:4, 0:256])
        yd4 = sb.tile([C, 4], fp16, tag="yd4")
        nc.sync.dma_start(out=yd4[:], in_=ydel_h.ap()[0:4, 256:260])
        b4 = sb.tile([C, 1], fp32, tag="b4")
        nc.sync.dma_start(out=b4[:], in_=b4_h.ap()[:])

        # persistent tiles
        aT = [sb.tile([128, D], fp16, tag=f"aT{k}", name=f"aT{k}") for k in range(4)]
        vt = sb.tile([C, A], fp16, tag="vt")  # e^{R'+b} transposed
        ut4 = sb.tile([C, RB], fp16, tag="ut4")  # e^{L} transposed
        yb = sb.tile([C, RB * C], fp16, tag="yb")  # L*delta over (i,c)
        x8 = sb.tile([8, A], fp16, tag="x8")  # rows 0-3 = R'+b, rows 4-7 = 1
        nc.gpsimd.memset(x8[:], 1.0)  # rows 0-3 overwritten with R'+b below
        lnse = sb.tile([128, RB * 4], fp32, tag="lnse")
        oj = sb.tile([128, RB * C * 4], fp32, tag="oj")

        # ---- transposes: acts[j][:, 128k:+128] -> aT[k][:, 128j:+128] ----
        prL = psR.tile([C, RB], fp32, tag="prL", name="prL")
        prR = psR.tile([C, A], fp32, tag="prR", name="prR")

        def do_chunk_transposes(j):
            for k in range(4):
                pt = psT.tile([128, 128], fp16, tag="pt", name="pt")
                nc.tensor.transpose(
                    out=pt[:],
                    in_=acts[j][:, 128 * k : 128 * k + 128],
                    identity=ident[:],
                )
                nc.vector.tensor_copy(
                    out=aT[k][:, 128 * j : 128 * j + 128], in_=pt[:]
                )

        do_chunk_transposes(0)
        # L^T [4, 64]: only needs chunk-0 columns of each aT[k]
        for k in range(4):
            nc.tensor.matmul(
                out=prL[:],
                lhsT=wt[:, 8 * k + 4 : 8 * k + 8],
                rhs=aT[k][:, 0:RB],
                start=(k == 0),
                stop=(k == 3),
            )
        for j in range(1, 4):
            do_chunk_transposes(j)
        # R'^T [4, 512] accumulated over the 4 d-chunks
        for k in range(4):
            nc.tensor.matmul(
                out=prR[:],
                lhsT=wt[:, 8 * k : 8 * k + 4],
                rhs=aT[k][:],
                start=(k == 0),
                stop=(k == 3),
            )

        # ---- activations + small combines (early L-side ones first) ----
        nc.scalar.activation(out=ut4[:], in_=prL[:], func=AF.Exp)
        # yb[c', (i,c)] = L^T[c', i] * delta[c', c], then DMA into y8 rows 4-7
        nc.vector.tensor_tensor(
            out=yb[:].rearrange("p (i c) -> p i c", c=C),
            in0=prL[:].unsqueeze(2).to_broadcast([C, RB, C]),
            in1=yd4[:].unsqueeze(1).to_broadcast([C, RB, C]),
            op=MULT,
        )
        nc.scalar.dma_start(out=y8[4:8, :], in_=yb[:])
        nc.scalar.activation(out=vt[:], in_=prR[:], func=AF.Exp, bias=b4[:])
        # x8 rows 0-3 = (R' + b) as fp16
        nc.vector.tensor_scalar_add(x8[0:4, :], prR[:], b4[:])

        # ---- phase B per j-chunk: lse, ln, M, subtract, store ----
        for j in range(4):
            se = psS.tile([128, RB], fp32, tag="se", name="se")
            nc.tensor.matmul(
                out=se[:],
                lhsT=vt[:, 128 * j : 128 * (j + 1)],
                rhs=ut4[:],
                start=True,
                stop=True,
            )
            nc.scalar.activation(
                out=lnse[:, RB * j : RB * (j + 1)], in_=se[:], func=AF.Ln
            )
            # M[j', (i,c)] = (R'+b)[128j+j', c] + L[i, c] in one K=8 matmul
            mj = psM.tile([128, RB * C], fp32, tag="mj", name="mj")
            nc.tensor.matmul(
                out=mj[:],
                lhsT=x8[:, 128 * j : 128 * (j + 1)],
                rhs=y8[:],
                start=True,
                stop=True,
            )
            nc.vector.tensor_tensor(
                out=oj[:, 256 * j : 256 * (j + 1)].rearrange(
                    "p (i c) -> p i c", c=C
                ),
                in0=mj[:].rearrange("p (i c) -> p i c", c=C),
                in1=lnse[:, RB * j : RB * (j + 1)]
                .unsqueeze(2)
                .to_broadcast([128, RB, C]),
                op=SUB,
            )
            eng = nc.sync if j % 2 == 0 else nc.scalar
            eng.dma_start(
                out=out_ap[128 * j : 128 * (j + 1), :],
                in_=oj[:, 256 * j : 256 * (j + 1)],
            )

    nc.compile()
    return nc


def _get_program():
    global _program
    if _program is None:
        _program = _build_program()
    return _program


def _prep_core_inputs(emb16, idx64, wt_np, b4_np, k):
    rot = np.roll(idx64, -RB * k)
    idxs = np.ascontiguousarray(rot.reshape(4, 128).T.astype(np.int32))
    return {"emb16": emb16, "idxs": idxs, "wt16": wt_np, "b4": b4_np}


def kernel(embeds, activity_index, W, b):
    from concourse.bass_utils import run_bass_kernel_spmd

    embeds = np.asarray(embeds, dtype=np.float32)
    emb16 = np.ascontiguousarray(embeds.astype(np.float16))
    W = np.asarray(W, dtype=np.float32)
    b_in = np.asarray(b, dtype=np.float32).reshape(C)
    idx64 = np.asarray(activity_index).astype(np.int64)

    # wt16[d, 8k+0:4] = Wr.T chunk k, wt16[d, 8k+4:8] = Wl.T chunk k
    wt_np = np.empty((128, 32), dtype=np.float16)
    for k in range(4):
        wt_np[:, 8 * k : 8 * k + 4] = W[:, D + 128 * k : D + 128 * (k + 1)].T
        wt_np[:, 8 * k + 4 : 8 * k + 8] = W[:, 128 * k : 128 * (k + 1)].T
    wt_np = np.ascontiguousarray(wt_np)
    b4_np = np.ascontiguousarray(b_in.reshape(C, 1))

    nc = _get_program()
    in_maps = [
        _prep_core_inputs(emb16, idx64, wt_np, b4_np, k) for k in range(NCORES)
    ]

    results = run_bass_kernel_spmd(nc, in_maps, core_ids=list(range(NCORES)))
    global _last_results
    _last_results = results

    out_sq = np.empty((A, A, C), dtype=np.float32)
    for k in range(NCORES):
        # blk[j, i, c] with j rotated by -64k -> un-rotate and transpose
        blk = results.results[k]["out"].reshape(A, RB, C).transpose(1, 0, 2)
        out_sq[RB * k : RB * (k + 1)] = np.roll(blk, RB * k, axis=1)

    ii, jj = np.triu_indices(A, k=1)
    return np.ascontiguousarray(out_sq[ii, jj])
